# revision 2
# baseline (speedup 1.0000x reference)
"""MinLSTM fused kernel for Trainium2 (8 NeuronCores, SPMD).

Math: the reference applies cumlogsumexp over the sequence but only the LAST
timestep feeds the output head, so the scan collapses to a single logsumexp
reduction over sequence:

    log_h_last = log_f[S-1] + log(0.5 + sum_s exp(diff_s + log_g(h_s)))
    out = exp(log_h_last) @ w_out.T + b_out

with diff = softplus(-f) - softplus(-i) and per-token term

    exp(diff + log_g(h)) = (1 + e^{-f}) * sigmoid(i) * g(h)
                         = 1/4 * (1+e^{-f}) * (1+tanh(i/2)) * (1+max(2h, tanh(h/2)))

which needs only {exp, tanh} — both in the ACT `exp_and_others` table set
(single table load). The device computes, per core, the partial sum over its
4096 tokens of that product for each of the 1024 hidden channels, fused with
the z = x @ w_in.T matmul (fp8 DoubleRow, fp32 PSUM accumulation). The host
combines partials, applies the exact last-token correction in fp64, and runs
the tiny [4,1024]x[1024,1024] output head.

The PE is the bottleneck: fp8 DoubleRow streams one 256-deep moving column
per cycle (measured 216 ns per FD=512 matmul — the 157 TF/s peak), so the
393216-cycle matmul stream (~164 us) is the floor. The first 512 tokens are
processed as two 256-token halves sharing PSUM banks so each weight load
serves both halves (small-FD matmuls are LDWEIGHTS-bound otherwise); the
redundant LDWEIGHTS that tile_legalize emits 1:1 with matmuls are rewritten
to no-ops post-schedule (dedup_ldweights).

Sharding: data-parallel over flattened (batch, seq) tokens — core c takes
tokens [c*4096, (c+1)*4096), i.e. batch c//2, sequence half c%2. The sum over
seq is order-independent, so partials combine by addition on host.
"""

from contextlib import ExitStack

import ml_dtypes
import numpy as np

B, S, D, H = 4, 8192, 1024, 1024
N_CORES = 8
TOK = B * S // N_CORES  # 4096 tokens per core
TB = 512                # token block (matmul moving free dim / PSUM bank)
NTB = TOK // TB         # 8 units (first = two packed 256-halves)
KC = D // 128           # 8 contraction chunks of 128
JC = H // 128           # 8 hidden-channel chunks per gate

WSCALE = 64.0           # w pre-scale so fp8 w values sit in the normal range

_CACHE = {}


def _dedup_ldweights(nc):
    """Rewrite InstLdweights whose weights AP + perf mode match the previous
    load into no-ops (same name + sync_info preserved), so the PE keeps the
    already-loaded weights. Run after move_matmul_waits_to_ldweights so each
    matmul's waits have already landed on its own (possibly nopped) LDW."""
    import concourse.mybir as mybir

    n_nopped = 0
    for blk in nc.main_func.blocks:
        last_sig = None
        for idx, inst in enumerate(blk.instructions):
            if isinstance(inst, mybir.InstLdweights):
                sig = (str(inst.ins[0]), str(inst.perf_mode))
                if sig == last_sig:
                    nop = mybir.InstNoOp(name=inst.name, ins=[], outs=[])
                    nop.engine = inst.engine
                    nop.sync_info = inst.sync_info
                    nc.register_instruction(nop, overwrite=True)
                    blk.instructions[idx] = nop
                    n_nopped += 1
                else:
                    last_sig = sig
    return n_nopped


def _build_nc():
    import concourse.bacc as bacc
    import concourse.mybir as mybir
    import concourse.tile as tile

    dt = mybir.dt
    AF = mybir.ActivationFunctionType
    ALU = mybir.AluOpType

    in_dt = dt.float8e4
    inv = 1.0 / WSCALE

    nc = bacc.Bacc("TRN2", target_bir_lowering=False)
    xT = nc.dram_tensor("xt", (D, TOK), in_dt, kind="ExternalInput")
    # wt[j, d, g*128+c] = w_in[g*H + j*128 + c, d] (pre-permuted on host so
    # each j-stripe is one contiguous 3-dim HWDGE DMA)
    wT = nc.dram_tensor("wt", (JC, D, 384), in_dt, kind="ExternalInput")
    # startup-critical first x half-block and first w stripe, host-packed in
    # exact SBUF partition-major layout so their DMAs are dense bursts. The
    # first w stripe is split into kb-pair chunks so the first matmul can
    # start after ~100KB instead of ~400KB.
    xf0 = nc.dram_tensor("xf0", (128, KC, 256), in_dt, kind="ExternalInput")
    wj0 = nc.dram_tensor("wj0", (KC // 2, 128, 2, 384), in_dt, kind="ExternalInput")
    # [partition, j] layout: h-channel = j*128 + p. Contiguous per partition
    # so the final DMA is not a 4-byte-scatter.
    out_sums = nc.dram_tensor("sums", (128, JC), dt.float32, kind="ExternalOutput")

    with tile.TileContext(nc) as tc, ExitStack() as ctx:
        wpool = ctx.enter_context(tc.tile_pool(name="w", bufs=1))
        xpool = ctx.enter_context(tc.tile_pool(name="x", bufs=3))
        gpool = ctx.enter_context(tc.tile_pool(name="g", bufs=3))
        spool = ctx.enter_context(tc.tile_pool(name="s", bufs=1))
        psum = ctx.enter_context(tc.tile_pool(name="psum", bufs=2, space="PSUM"))

        slab = spool.tile([128, JC, NTB], dt.float32)

        xT_r = xT[:].rearrange("(kc p) s -> p kc s", p=128)

        # preload x for the first 512 tokens (two 256-halves: xa dense-packed
        # from host, xb strided on the scalar queue), then stream w in
        # j-ordered stripes so the first matmul group only waits for the
        # first kb-chunk of j0's weights.
        xa = xpool.tile([128, KC, 256], in_dt, tag="x0")
        nc.sync.dma_start(xa[:], xf0[:])
        xb = xpool.tile([128, KC, 256], in_dt, tag="xb")
        w_all = wpool.tile([128, KC, JC * 384], in_dt)
        wT_j = [wT[j].rearrange("(kc p) gc -> p kc gc", p=128) for j in range(JC)]
        for k in range(KC // 2):
            nc.scalar.dma_start(w_all[:, 2 * k : 2 * k + 2, 0:384], wj0[k])
        nc.scalar.dma_start(xb[:], xT_r[:, :, 256:512])
        for j in range(1, JC):
            nc.sync.dma_start(w_all[:, :, j * 384 : (j + 1) * 384], wT_j[j])

        for tb in range(NTB):
            toff = tb * TB
            if tb == 0:
                x_pair = (xa, xb)
                x_sb = None
            else:
                x_pair = None
                x_sb = xpool.tile([128, KC, TB], in_dt, tag="x1")
                nc.sync.dma_start(x_sb[:], xT_r[:, :, toff : toff + TB])
            for j in range(JC):
                # f-gate in its own bank; i and h share a 2-bank tile so one
                # FD=1024 tanh ACTIVATE covers both. 2 + 3*2 = 8 PSUM banks.
                ps0 = psum.tile([128, TB], dt.float32, tag="ps0", bufs=2)
                ps12 = psum.tile([128, 2, TB], dt.float32, tag="ps12", bufs=3)

                def mm_group(out_ap, hs):
                    for kb in range(KC // 2):
                        nc.tensor.matmul(
                            out_ap,
                            w_all[:, 2 * kb : 2 * kb + 2, hs : hs + 128],
                            x_sb[:, 2 * kb : 2 * kb + 2, :],
                            start=(kb == 0),
                            stop=(kb == KC // 2 - 1),
                            perf_mode=mybir.MatmulPerfMode.DoubleRow,
                        )

                def mm_group_pair(out_ap, hs):
                    # two 256-token halves accumulated in one bank with a
                    # single weight load per kb chunk (the duplicate LDW for
                    # the second half is nopped by _dedup_ldweights). Only
                    # the very first matmul clears the bank's has_written
                    # bits; the first half-1 matmul then overwrites its
                    # still-clear region and later chunks accumulate.
                    for kb in range(KC // 2):
                        for half, xh in enumerate(x_pair):
                            nc.tensor.matmul(
                                out_ap[:, half * 256 : (half + 1) * 256]
                                if out_ap.ndim == 2
                                else out_ap[:, :, half * 256 : (half + 1) * 256],
                                w_all[:, 2 * kb : 2 * kb + 2, hs : hs + 128],
                                xh[:, 2 * kb : 2 * kb + 2, :],
                                start=(kb == 0 and half == 0),
                                stop=(kb == KC // 2 - 1 and half == 1),
                                perf_mode=mybir.MatmulPerfMode.DoubleRow,
                                skip_group_check=True,
                            )

                if tb == 0:
                    mm_group_pair(ps0[:], j * 384)
                    mm_group_pair(ps12[:, 0, :], j * 384 + 128)
                    mm_group_pair(ps12[:, 1, :], j * 384 + 256)
                else:
                    mm_group(ps0[:], j * 384)
                    mm_group(ps12[:, 0, :], j * 384 + 128)
                    mm_group(ps12[:, 1, :], j * 384 + 256)

                a = gpool.tile([128, TB], dt.bfloat16, tag="a")
                tith = gpool.tile([128, 2, TB], dt.bfloat16, tag="tith")
                # tanh first: the DVE chain consumes tith immediately but a
                # only at its last op, and this frees the 2-bank tile sooner.
                nc.scalar.activation(tith[:], ps12[:], AF.Tanh, scale=0.5 * inv)
                nc.scalar.activation(a[:], ps0[:], AF.Exp, scale=-inv)
                # m1 = max(2h, tanh(h/2));  p = (1+tanh(i/2)) * (1+m1)
                m1 = gpool.tile([128, TB], dt.bfloat16, tag="m1")
                nc.vector.scalar_tensor_tensor(
                    m1[:], ps12[:, 1, :], 2.0 * inv, tith[:, 1, :],
                    op0=ALU.mult, op1=ALU.max,
                )
                w2 = gpool.tile([128, TB], dt.bfloat16, tag="w2")
                nc.vector.tensor_scalar_add(w2[:], m1[:], 1.0)
                p = gpool.tile([128, TB], dt.bfloat16, tag="p")
                nc.vector.scalar_tensor_tensor(
                    p[:], tith[:, 0, :], 1.0, w2[:], op0=ALU.add, op1=ALU.mult
                )
                # t = (1+e^{-f}) * p, accumulated over the 512 tokens
                t = gpool.tile([128, TB], dt.bfloat16, tag="t")
                nc.vector.scalar_tensor_tensor(
                    t[:],
                    a[:],
                    1.0,
                    p[:],
                    op0=ALU.add,
                    op1=ALU.mult,
                    accum_out=slab[:, j, tb : tb + 1],
                )

        red = spool.tile([128, JC], dt.float32)
        nc.vector.tensor_reduce(red[:], slab[:], axis=mybir.AxisListType.X, op=ALU.add)
        nc.sync.dma_start(out_sums[:], red[:])

    nc.move_matmul_waits_to_ldweights()
    _dedup_ldweights(nc)
    nc.compile()
    return nc


def _get_nc():
    if "v2" not in _CACHE:
        _CACHE["v2"] = _build_nc()
    return _CACHE["v2"]


def _softplus(v):
    return np.log1p(np.exp(-np.abs(v))) + np.maximum(v, 0.0)


def kernel(x, w_in, w_out, b_out, _return_results=False, _trace=False):
    from concourse.bass_utils import run_bass_kernel_spmd

    x = np.asarray(x)
    w_in = np.asarray(w_in)
    w_out = np.asarray(w_out)
    b_out = np.asarray(b_out)

    cast_dt = ml_dtypes.float8_e4m3  # TRN FP8_EXP4: max ±240, inf above

    def cast(a):
        return np.clip(a, -240.0, 240.0).astype(cast_dt)

    w_scaled = w_in * WSCALE
    # wt[j, d, g*128+c] = w_scaled[g*H + j*128 + c, d]
    wT = cast(
        np.ascontiguousarray(
            w_scaled.reshape(3, JC, 128, D).transpose(1, 3, 0, 2).reshape(JC, D, 384)
        )
    )

    # first w stripe packed [kb-pair, p, 2, 384] to match the SBUF tile slices
    wj0 = np.ascontiguousarray(
        np.asarray(wT[0]).reshape(KC // 2, 2, 128, 384).transpose(0, 2, 1, 3)
    )
    xf = x.reshape(B * S, D)
    in_maps = []
    for c in range(N_CORES):
        xs = xf[c * TOK : (c + 1) * TOK]  # [TOK, D]
        xt = cast(np.ascontiguousarray(xs.T))  # [D, TOK]
        xf0 = np.ascontiguousarray(
            np.asarray(xt).reshape(KC, 128, TOK)[:, :, :256].transpose(1, 0, 2)
        )
        in_maps.append({"xt": xt, "wt": wT, "xf0": xf0, "wj0": wj0})

    nc = _get_nc()
    # the first execution of a freshly compiled NEFF occasionally hits a
    # transient NRT exec error on this setup — retry once
    try:
        res = run_bass_kernel_spmd(
            nc, in_maps, core_ids=list(range(N_CORES)), trace=_trace
        )
    except Exception:
        import time as _time

        _time.sleep(2.0)
        res = run_bass_kernel_spmd(
            nc, in_maps, core_ids=list(range(N_CORES)), trace=False
        )

    # sums[p, j] -> channel h = j*128 + p
    parts = [
        np.asarray(r["sums"]).T.reshape(H).astype(np.float64) for r in res.results
    ]
    Ssum = np.stack([parts[2 * b] + parts[2 * b + 1] for b in range(B)]) * 0.25

    # exact last-token factor in fp64 (host): log_f[S-1] = -softplus(diff[S-1])
    z_last = x[:, -1, :].astype(np.float64) @ w_in.astype(np.float64).T
    f_l, i_l = z_last[:, :H], z_last[:, H : 2 * H]
    diff_l = _softplus(-f_l) - _softplus(-i_l)
    h_last = np.exp(-_softplus(diff_l) + np.log(0.5 + Ssum))
    out = (h_last @ w_out.astype(np.float64).T + b_out.astype(np.float64)).astype(
        np.float32
    )
    if _return_results:
        return out, res
    return out


# revision 5
# speedup vs baseline: 1.0145x; 1.0145x over previous
"""MinLSTM fused kernel for Trainium2 (8 NeuronCores, SPMD).

Math: the reference applies cumlogsumexp over the sequence but only the LAST
timestep feeds the output head, so the scan collapses to a single logsumexp
reduction over sequence:

    log_h_last = log_f[S-1] + log(0.5 + sum_s exp(diff_s + log_g(h_s)))
    out = exp(log_h_last) @ w_out.T + b_out

with diff = softplus(-f) - softplus(-i) and per-token term

    exp(diff + log_g(h)) = (1 + e^{-f}) * sigmoid(i) * g(h)
                         = 1/4 * (1+e^{-f}) * (1+tanh(i/2)) * (1+max(2h, tanh(h/2)))

which needs only {exp, tanh} — both in the ACT `exp_and_others` table set
(single table load). The device computes, per core, the partial sum over its
4096 tokens of that product for each of the 1024 hidden channels, fused with
the z = x @ w_in.T matmul (fp8 DoubleRow, fp32 PSUM accumulation). The host
combines partials, applies the exact last-token correction in fp64, and runs
the tiny [4,1024]x[1024,1024] output head.

The PE is the bottleneck: fp8 DoubleRow streams one 256-deep moving column
per cycle (measured 216 ns per FD=512 matmul — the 157 TF/s peak), so the
393216-cycle matmul stream (~164 us) is the floor. The first 512 tokens are
processed as two 256-token halves sharing PSUM banks so each weight load
serves both halves (small-FD matmuls are LDWEIGHTS-bound otherwise); the
redundant LDWEIGHTS that tile_legalize emits 1:1 with matmuls are rewritten
to no-ops post-schedule (dedup_ldweights).

Sharding: data-parallel over flattened (batch, seq) tokens — core c takes
tokens [c*4096, (c+1)*4096), i.e. batch c//2, sequence half c%2. The sum over
seq is order-independent, so partials combine by addition on host.
"""

from contextlib import ExitStack

import ml_dtypes
import numpy as np

B, S, D, H = 4, 8192, 1024, 1024
N_CORES = 8
TOK = B * S // N_CORES  # 4096 tokens per core
TB = 512                # token block (matmul moving free dim / PSUM bank)
NTB = TOK // TB         # 8 units (first = two packed 256-halves)
KC = D // 128           # 8 contraction chunks of 128
JC = H // 128           # 8 hidden-channel chunks per gate

WSCALE = 64.0           # w pre-scale so fp8 w values sit in the normal range

_CACHE = {}


def _dedup_ldweights(nc):
    """Rewrite InstLdweights whose weights AP + perf mode match the previous
    load into no-ops (same name + sync_info preserved), so the PE keeps the
    already-loaded weights. Run after move_matmul_waits_to_ldweights so each
    matmul's waits have already landed on its own (possibly nopped) LDW."""
    import concourse.mybir as mybir

    n_nopped = 0
    for blk in nc.main_func.blocks:
        last_sig = None
        for idx, inst in enumerate(blk.instructions):
            if isinstance(inst, mybir.InstLdweights):
                sig = (str(inst.ins[0]), str(inst.perf_mode))
                if sig == last_sig:
                    nop = mybir.InstNoOp(name=inst.name, ins=[], outs=[])
                    nop.engine = inst.engine
                    nop.sync_info = inst.sync_info
                    nc.register_instruction(nop, overwrite=True)
                    blk.instructions[idx] = nop
                    n_nopped += 1
                else:
                    last_sig = sig
    return n_nopped


def _build_nc():
    import concourse.bacc as bacc
    import concourse.mybir as mybir
    import concourse.tile as tile

    dt = mybir.dt
    AF = mybir.ActivationFunctionType
    ALU = mybir.AluOpType

    in_dt = dt.float8e4
    inv = 1.0 / WSCALE

    nc = bacc.Bacc("TRN2", target_bir_lowering=False)
    xT = nc.dram_tensor("xt", (D, TOK), in_dt, kind="ExternalInput")
    # wt[j, d, g*128+c] = w_in[g*H + j*128 + c, d] (pre-permuted on host so
    # each j-stripe is one contiguous 3-dim HWDGE DMA)
    wT = nc.dram_tensor("wt", (JC, D, 384), in_dt, kind="ExternalInput")
    # startup-critical first x half-block and first w stripe, host-packed in
    # exact SBUF partition-major layout so their DMAs are dense bursts. The
    # first w stripe is split into kb-pair chunks so the first matmul can
    # start after ~100KB instead of ~400KB.
    xf0 = nc.dram_tensor("xf0", (128, KC, 256), in_dt, kind="ExternalInput")
    xf1 = nc.dram_tensor("xf1", (128, KC, 256), in_dt, kind="ExternalInput")
    wj0 = nc.dram_tensor("wj0", (KC // 2, 128, 2, 384), in_dt, kind="ExternalInput")
    # [partition, j] layout: h-channel = j*128 + p. Contiguous per partition
    # so the final DMA is not a 4-byte-scatter.
    out_sums = nc.dram_tensor("sums", (128, JC), dt.float32, kind="ExternalOutput")

    with tile.TileContext(nc) as tc, ExitStack() as ctx:
        wpool = ctx.enter_context(tc.tile_pool(name="w", bufs=1))
        xpool = ctx.enter_context(tc.tile_pool(name="x", bufs=3))
        gpool = ctx.enter_context(tc.tile_pool(name="g", bufs=3))
        spool = ctx.enter_context(tc.tile_pool(name="s", bufs=1))
        psum = ctx.enter_context(tc.tile_pool(name="psum", bufs=2, space="PSUM"))

        slab = spool.tile([128, JC, NTB], dt.float32)

        xT_r = xT[:].rearrange("(kc p) s -> p kc s", p=128)

        # preload x for the first 512 tokens (two 256-halves: xa dense-packed
        # from host, xb strided on the scalar queue), then stream w in
        # j-ordered stripes so the first matmul group only waits for the
        # first kb-chunk of j0's weights.
        xa = xpool.tile([128, KC, 256], in_dt, tag="x0")
        nc.sync.dma_start(xa[:], xf0[:])
        xb = xpool.tile([128, KC, 256], in_dt, tag="xb")
        w_all = wpool.tile([128, KC, JC * 384], in_dt)
        wT_j = [wT[j].rearrange("(kc p) gc -> p kc gc", p=128) for j in range(JC)]
        nc.scalar.dma_start(w_all[:, 0:2, 0:384], wj0[0])
        nc.scalar.dma_start(xb[:], xf1[:])
        for k in range(1, KC // 2):
            nc.scalar.dma_start(w_all[:, 2 * k : 2 * k + 2, 0:384], wj0[k])
        for j in range(1, JC):
            nc.sync.dma_start(w_all[:, :, j * 384 : (j + 1) * 384], wT_j[j])

        for tb in range(NTB):
            toff = tb * TB
            if tb == 0:
                x_pair = (xa, xb)
                x_sb = None
            else:
                x_pair = None
                x_sb = xpool.tile([128, KC, TB], in_dt, tag="x1")
                nc.sync.dma_start(x_sb[:], xT_r[:, :, toff : toff + TB])
            for j in range(JC):
                # f-gate in its own bank; i and h share a 2-bank tile so one
                # FD=1024 tanh ACTIVATE covers both. 2 + 3*2 = 8 PSUM banks.
                ps0 = psum.tile([128, TB], dt.float32, tag="ps0", bufs=2)
                ps12 = psum.tile([128, 2, TB], dt.float32, tag="ps12", bufs=3)

                def mm_group(out_ap, hs):
                    for kb in range(KC // 2):
                        nc.tensor.matmul(
                            out_ap,
                            w_all[:, 2 * kb : 2 * kb + 2, hs : hs + 128],
                            x_sb[:, 2 * kb : 2 * kb + 2, :],
                            start=(kb == 0),
                            stop=(kb == KC // 2 - 1),
                            perf_mode=mybir.MatmulPerfMode.DoubleRow,
                        )

                def mm_group_pair(out_ap, hs):
                    # two 256-token halves accumulated in one bank with a
                    # single weight load per kb chunk (the duplicate LDW for
                    # the second half is nopped by _dedup_ldweights). Only
                    # the very first matmul clears the bank's has_written
                    # bits; the first half-1 matmul then overwrites its
                    # still-clear region and later chunks accumulate.
                    for kb in range(KC // 2):
                        for half, xh in enumerate(x_pair):
                            nc.tensor.matmul(
                                out_ap[:, half * 256 : (half + 1) * 256]
                                if out_ap.ndim == 2
                                else out_ap[:, :, half * 256 : (half + 1) * 256],
                                w_all[:, 2 * kb : 2 * kb + 2, hs : hs + 128],
                                xh[:, 2 * kb : 2 * kb + 2, :],
                                start=(kb == 0 and half == 0),
                                stop=(kb == KC // 2 - 1 and half == 1),
                                perf_mode=mybir.MatmulPerfMode.DoubleRow,
                                skip_group_check=True,
                            )

                if tb == 0:
                    mm_group_pair(ps0[:], j * 384)
                    mm_group_pair(ps12[:, 0, :], j * 384 + 128)
                    mm_group_pair(ps12[:, 1, :], j * 384 + 256)
                else:
                    mm_group(ps0[:], j * 384)
                    mm_group(ps12[:, 0, :], j * 384 + 128)
                    mm_group(ps12[:, 1, :], j * 384 + 256)

                a = gpool.tile([128, TB], dt.bfloat16, tag="a")
                tith = gpool.tile([128, 2, TB], dt.bfloat16, tag="tith")
                # tanh first: the DVE chain consumes tith immediately but a
                # only at its last op, and this frees the 2-bank tile sooner.
                nc.scalar.activation(tith[:], ps12[:], AF.Tanh, scale=0.5 * inv)
                nc.scalar.activation(a[:], ps0[:], AF.Exp, scale=-inv)
                # m1 = max(2h, tanh(h/2));  p = (1+tanh(i/2)) * (1+m1)
                m1 = gpool.tile([128, TB], dt.bfloat16, tag="m1")
                nc.vector.scalar_tensor_tensor(
                    m1[:], ps12[:, 1, :], 2.0 * inv, tith[:, 1, :],
                    op0=ALU.mult, op1=ALU.max,
                )
                w2 = gpool.tile([128, TB], dt.bfloat16, tag="w2")
                nc.vector.tensor_scalar_add(w2[:], m1[:], 1.0)
                p = gpool.tile([128, TB], dt.bfloat16, tag="p")
                nc.vector.scalar_tensor_tensor(
                    p[:], tith[:, 0, :], 1.0, w2[:], op0=ALU.add, op1=ALU.mult
                )
                # t = (1+e^{-f}) * p, accumulated over the 512 tokens
                t = gpool.tile([128, TB], dt.bfloat16, tag="t")
                nc.vector.scalar_tensor_tensor(
                    t[:],
                    a[:],
                    1.0,
                    p[:],
                    op0=ALU.add,
                    op1=ALU.mult,
                    accum_out=slab[:, j, tb : tb + 1],
                )

        red = spool.tile([128, JC], dt.float32)
        nc.vector.tensor_reduce(red[:], slab[:], axis=mybir.AxisListType.X, op=ALU.add)
        nc.sync.dma_start(out_sums[:], red[:])

    nc.move_matmul_waits_to_ldweights()
    _dedup_ldweights(nc)
    nc.compile()
    return nc


def _get_nc():
    if "v2" not in _CACHE:
        _CACHE["v2"] = _build_nc()
    return _CACHE["v2"]


def _softplus(v):
    return np.log1p(np.exp(-np.abs(v))) + np.maximum(v, 0.0)


def kernel(x, w_in, w_out, b_out, _return_results=False, _trace=False):
    from concourse.bass_utils import run_bass_kernel_spmd

    x = np.asarray(x)
    w_in = np.asarray(w_in)
    w_out = np.asarray(w_out)
    b_out = np.asarray(b_out)

    cast_dt = ml_dtypes.float8_e4m3  # TRN FP8_EXP4: max ±240, inf above

    def cast(a):
        return np.clip(a, -240.0, 240.0).astype(cast_dt)

    w_scaled = w_in * WSCALE
    # wt[j, d, g*128+c] = w_scaled[g*H + j*128 + c, d]
    wT = cast(
        np.ascontiguousarray(
            w_scaled.reshape(3, JC, 128, D).transpose(1, 3, 0, 2).reshape(JC, D, 384)
        )
    )

    # first w stripe packed [kb-pair, p, 2, 384] to match the SBUF tile slices
    wj0 = np.ascontiguousarray(
        np.asarray(wT[0]).reshape(KC // 2, 2, 128, 384).transpose(0, 2, 1, 3)
    )
    xf = x.reshape(B * S, D)
    in_maps = []
    for c in range(N_CORES):
        xs = xf[c * TOK : (c + 1) * TOK]  # [TOK, D]
        xt = cast(np.ascontiguousarray(xs.T))  # [D, TOK]
        xtr = np.asarray(xt).reshape(KC, 128, TOK)
        xf0 = np.ascontiguousarray(xtr[:, :, :256].transpose(1, 0, 2))
        xf1 = np.ascontiguousarray(xtr[:, :, 256:512].transpose(1, 0, 2))
        in_maps.append({"xt": xt, "wt": wT, "xf0": xf0, "xf1": xf1, "wj0": wj0})

    nc = _get_nc()
    # the first execution of a freshly compiled NEFF occasionally hits a
    # transient NRT exec error on this setup — retry once
    try:
        res = run_bass_kernel_spmd(
            nc, in_maps, core_ids=list(range(N_CORES)), trace=_trace
        )
    except Exception:
        import time as _time

        _time.sleep(2.0)
        res = run_bass_kernel_spmd(
            nc, in_maps, core_ids=list(range(N_CORES)), trace=False
        )

    # sums[p, j] -> channel h = j*128 + p
    parts = [
        np.asarray(r["sums"]).T.reshape(H).astype(np.float64) for r in res.results
    ]
    Ssum = np.stack([parts[2 * b] + parts[2 * b + 1] for b in range(B)]) * 0.25

    # exact last-token factor in fp64 (host): log_f[S-1] = -softplus(diff[S-1])
    z_last = x[:, -1, :].astype(np.float64) @ w_in.astype(np.float64).T
    f_l, i_l = z_last[:, :H], z_last[:, H : 2 * H]
    diff_l = _softplus(-f_l) - _softplus(-i_l)
    h_last = np.exp(-_softplus(diff_l) + np.log(0.5 + Ssum))
    out = (h_last @ w_out.astype(np.float64).T + b_out.astype(np.float64)).astype(
        np.float32
    )
    if _return_results:
        return out, res
    return out
